# revision 4
# baseline (speedup 1.0000x reference)
"""Trainium2 Bass kernel for nn_Encoder_Decoder_fc (encoder LSTM -> decoder
LSTMCell + Linear), transposed-gate formulation.

Data-parallel over batch (B=256 -> 32 per core on 8 cores), weights replicated.
Per step the gate pre-activations are computed TRANSPOSED:
  PSUM G[p, c*128 + 32*j + b] = gate_c_pre[b, 128*j + p]
  (c: 0=i,1=f,2=g,3=o; j: h-chunk 0..3; b: batch 0..31)
Each recurrent matmul uses a full [128,128] stationary Whh^T tile and streams
the 32-wide h^T slab chunk, so PE streaming cost is 4x lower than a folded
(batch-partition) layout, and h^T comes out of the elementwise ops directly in
matmul-operand layout -- no PE transposes at all.

Schedule notes (tuned against the Tile cost model):
- gates land in two PSUM banks, [i|f|g'] and [o]; the g-gate weights are
  pre-scaled by 2 on the host so tanh(g) = 2*sigmoid(2g)-1 comes out of the
  SAME sigmoid ACT op as i and f (one big ACT op instead of two);
- the cell state is tracked as ct = c/2, making the update
  ct = sf*ct + si*(sigmoid(2g)-0.5) four plain TensorTensor stages on GPSIMD
  (which cannot touch PSUM or run fused TensorScalarPtr on real HW); the /2
  is undone for free via scale=2 inside the tanh(c) ACT op;
- tanh(c) -> h^T -> next step's k-group matmuls are pipelined in 32-wide
  h-chunks: chunk 0's matmuls start while chunk 1 is still in the ACT queue;
- x-part matmuls (PSUM-initializing) prefetch into the idle PE window of the
  previous step's elementwise phase;
- the decoder Linear bias is a scalar and is added on the host.
"""

import sys

sys.path.insert(0, "/opt/trn_rl_repo")

from contextlib import ExitStack

import ml_dtypes
import numpy as np

import concourse.bass as bass
import concourse.mybir as mybir
import concourse.tile as tile
from concourse import bacc
from concourse.bass_utils import run_bass_kernel_spmd

P = 128
H = 512
B = 256
T = 512
N_CORES = 8
BL = B // N_CORES  # 32 batch per core
KC = H // P  # 4 h-chunks
NT = 16  # gate tiles per step (4 gates x 4 chunks)
RING = 32  # h^T ring slabs (2 windows of 16)
WIN = 16  # ys window size (steps)

F32 = mybir.dt.float32
BF16 = mybir.dt.bfloat16
AF = mybir.ActivationFunctionType
_MMDT = {"bf16": BF16, "fp32": F32}


def _step(nc, pools, consts, t_abs, prev_slab, cur_slab, sW, sU, c_tile,
          first_step, skip_main, mmdt, hsplit, csplit, parity=0):
    """One LSTM step in transposed layout."""
    ring, sXT = consts["ring"], consts["XT"]

    # two PSUM banks: [i|f|g'] and [o].  The g-gate weights are pre-scaled by
    # 2 on the host so tanh(g) = 2*sigmoid(g') - 1 comes out of the SAME
    # sigmoid op as i and f (one big ACT op instead of two).
    Gifg = pools["gif"][parity].tile([P, 3 * P], F32, tag="Gifg", name="Gifg")
    Go = pools["ggo"][parity].tile([P, P], F32, tag="Go", name="Go")
    def gslice(cj):
        t_, o_ = (Gifg, cj) if cj < 12 else (Go, cj - 12)
        return t_[:, 32 * o_ : 32 * (o_ + 1)]
    xt = sXT[:, t_abs * BL : (t_abs + 1) * BL]
    # PSUM zero-region semantics: only the FIRST matmul into each bank carries
    # start=True and only the LAST carries stop=True.
    for cj in range(NT):
        nc.tensor.matmul(
            gslice(cj),
            sU[:, P * cj : P * (cj + 1)],
            xt,
            start=(cj in (0, 12)),
            stop=(skip_main and cj in (11, NT - 1)),
            skip_group_check=True,
        )
    if not skip_main:
        slab = ring[:, prev_slab * P : (prev_slab + 1) * P]
        for k in range(KC):
            rhs = slab[:, 32 * k : 32 * (k + 1)]
            for cj in range(NT):
                nc.tensor.matmul(
                    gslice(cj),
                    sW[k][:, P * cj : P * (cj + 1)],
                    rhs,
                    start=False,
                    stop=(k == KC - 1 and cj in (11, NT - 1)),
                    skip_group_check=True,
                )

    # activations into separate PSUM tiles (exact deps, cheap PSUM access)
    sifg = pools["sif"].tile([P, 3 * P], F32, tag="sifg")
    nc.scalar.activation(sifg, Gifg, AF.Sigmoid)
    sif = sifg[:, 0 : 2 * P]
    sg = sifg[:, 2 * P : 3 * P]  # sigmoid(2g); tanh(g) = 2*sg - 1
    so = pools["so"].tile([P, P], F32, tag="so")
    nc.scalar.activation(so, Go, AF.Sigmoid)

    # c update on Pool, chunked csplit-ways. The kernel tracks ct = c/2, so
    #   ct = sf*ct + si*(sg - 0.5)        [since si*tanh(g)/2 = si*(sg-0.5)]
    # in 4 plain TensorTensor stages (walrus rejects fused TensorScalarPtr on
    # Pool). tanh(c) later reads ct with scale=2 inside the ACT op -- free.
    # u/t1h are chunk-wide tiles shared across chunks, which also pins the
    # scheduler to a chunk-major order so chunk 0 finishes first.
    cw = P // csplit
    half = consts["HALF"]
    u = pools["u"].tile([P, cw], F32, tag="u")
    t1h = pools["t1"].tile([P, cw], F32, tag="t1h")
    if first_step:
        for s in range(csplit):
            sl = slice(s * cw, (s + 1) * cw)
            nc.gpsimd.tensor_sub(u, sg[:, sl], half[:, 0:cw])
            nc.gpsimd.tensor_mul(c_tile[:, sl], sif[:, sl], u)
    else:
        for s in range(csplit):
            sl = slice(s * cw, (s + 1) * cw)
            nc.gpsimd.tensor_sub(u, sg[:, sl], half[:, 0:cw])
            nc.gpsimd.tensor_mul(c_tile[:, sl], sif[:, P + s * cw : P + (s + 1) * cw], c_tile[:, sl])
            nc.gpsimd.tensor_mul(t1h, sif[:, sl], u)
            nc.gpsimd.tensor_add(c_tile[:, sl], c_tile[:, sl], t1h)

    # chunked tail: tanh(c) -> h^T slab per chunk-group; two alternating PSUM
    # tiles so chunk s+2's write never waits on chunk s's reader
    slab_out = ring[:, cur_slab * P : (cur_slab + 1) * P]
    w = P // hsplit
    tcts = [
        pools["tcA"].tile([P, w], F32, tag="tcA", name="tcA"),
        pools["tcB"].tile([P, w], F32, tag="tcB", name="tcB"),
    ]
    for s in range(hsplit):
        sl = slice(s * w, (s + 1) * w)
        tc = tcts[s % 2][:, 0:w]
        nc.scalar.activation(tc, c_tile[:, sl], AF.Tanh, scale=2.0)
        nc.gpsimd.tensor_mul(slab_out[:, sl], so[:, sl], tc)


def _ys_window(nc, pools, consts, w, dY, nsteps=WIN):
    """Apply Linear to the h^T slabs of decoder window w and DMA the ys out.

    No bias here -- lin_b is added on the host."""
    ypool = pools["y"]
    ring5 = consts["ring5"]  # ring viewed [P, 2, WIN, KC, BL]
    sLW = consts["LW"]
    half = w % 2
    yps = ypool.tile([1, WIN * BL], F32, tag="yps")
    for k in range(KC):
        nc.tensor.matmul(
            yps[0:1, 0 : nsteps * BL],
            sLW[:, k : k + 1],
            ring5[:, half, 0:nsteps, k, :],
            start=(k == 0),
            stop=(k == KC - 1),
        )
    ysb = pools["ysb"].tile([1, WIN * BL], F32, tag="ysb")
    nc.vector.tensor_copy(ysb[0:1, 0 : nsteps * BL], yps[0:1, 0 : nsteps * BL])
    nc.sync.dma_start(
        dY[0:1, w * WIN * BL : w * WIN * BL + nsteps * BL],
        ysb[0:1, 0 : nsteps * BL],
    )


def build_nc(t_enc=T, t_dec=T, mm_dtype="bf16", hsplit=4, csplit=4, gbufs=1):
    mmdt = _MMDT[mm_dtype]
    nc = bacc.Bacc()

    dXT = nc.declare_dram_parameter("XT", [P, max(t_enc, t_dec) * BL], mmdt, isOutput=False)
    dWE = nc.declare_dram_parameter("WE", [KC, P, NT * P], mmdt, isOutput=False)
    dWD = nc.declare_dram_parameter("WD", [KC, P, NT * P], mmdt, isOutput=False)
    dUE = nc.declare_dram_parameter("UE", [P, NT * P], mmdt, isOutput=False)
    dUD = nc.declare_dram_parameter("UD", [P, NT * P], mmdt, isOutput=False)
    dLW = nc.declare_dram_parameter("LW", [P, KC], mmdt, isOutput=False)
    dHF = nc.declare_dram_parameter("HF", [P, P], F32, isOutput=False)
    dY = nc.declare_dram_parameter("Y", [1, t_dec * BL], F32, isOutput=True)

    with ExitStack() as ctx:
        tc_ctx = ctx.enter_context(tile.TileContext(nc))
        const = ctx.enter_context(tc_ctx.tile_pool(name="const", bufs=1))
        gifp0 = ctx.enter_context(tc_ctx.tile_pool(name="gif", bufs=1, space="PSUM"))
        ggop0 = ctx.enter_context(tc_ctx.tile_pool(name="ggo", bufs=1, space="PSUM"))
        gifp = [gifp0, gifp0]
        ggop = [ggop0, ggop0]
        ypool = ctx.enter_context(tc_ctx.tile_pool(name="yps", bufs=1, space="PSUM"))
        pscon = ctx.enter_context(tc_ctx.tile_pool(name="psc", bufs=1))
        # sig/tanh outputs live in SBUF: GPSIMD (which does the elementwise
        # c/h math) cannot access PSUM on real hardware
        sifp = ctx.enter_context(tc_ctx.tile_pool(name="sifp", bufs=2))
        sop = ctx.enter_context(tc_ctx.tile_pool(name="sop", bufs=2))
        tcap = ctx.enter_context(tc_ctx.tile_pool(name="tcap", bufs=1))
        tcbp = ctx.enter_context(tc_ctx.tile_pool(name="tcbp", bufs=1))
        t1p = ctx.enter_context(tc_ctx.tile_pool(name="t1p", bufs=2))
        up = ctx.enter_context(tc_ctx.tile_pool(name="up", bufs=2))
        ysbp = ctx.enter_context(tc_ctx.tile_pool(name="ysb", bufs=2))

        # persistent SBUF tensors
        sXT = const.tile([P, max(t_enc, t_dec) * BL], mmdt, tag="sXT")
        sWE = [
            const.tile([P, NT * P], mmdt, tag=f"sWE{k}", name=f"sWE{k}")
            for k in range(KC)
        ]
        sWD = [
            const.tile([P, NT * P], mmdt, tag=f"sWD{k}", name=f"sWD{k}")
            for k in range(KC)
        ]
        sUE = const.tile([P, NT * P], mmdt, tag="sUE")
        sUD = const.tile([P, NT * P], mmdt, tag="sUD")
        sLW = const.tile([P, KC], mmdt, tag="sLW")
        ring = const.tile([P, RING * P], mmdt, tag="ring")
        sHF = const.tile([P, P], F32, tag="sHF")
        c_tile = pscon.tile([P, P], F32, tag="c")

        nc.sync.dma_start(sXT[:, :], dXT[:, :])
        for k in range(KC):
            nc.sync.dma_start(sWE[k][:, :], dWE[k])
            nc.sync.dma_start(sWD[k][:, :], dWD[k])
        nc.sync.dma_start(sUE[:, :], dUE[:, :])
        nc.sync.dma_start(sUD[:, :], dUD[:, :])
        nc.sync.dma_start(sLW[:, :], dLW[:, :])
        nc.sync.dma_start(sHF[:, :], dHF[:, :])

        ring5 = ring.rearrange("p (u s k b) -> p u s k b", u=2, s=WIN, k=KC)
        pools = {
            "gif": gifp, "ggo": ggop, "sif": sifp, "so": sop, "tcA": tcap, "tcB": tcbp,
            "t1": t1p, "u": up, "y": ypool, "ysb": ysbp,
        }
        consts = {"ring": ring, "ring5": ring5, "XT": sXT, "LW": sLW,
                  "HALF": sHF}

        # ---------------- encoder ----------------
        for t in range(t_enc):
            _step(nc, pools, consts, t, (t - 1) % RING, t % RING, sWE, sUE,
                  c_tile, first_step=(t == 0), skip_main=(t == 0),
                  mmdt=mmdt, hsplit=hsplit, csplit=csplit, parity=t % 2)

        # ---------------- decoder ----------------
        enc_final = (t_enc - 1) % RING
        for t in range(t_dec):
            prev = enc_final if t == 0 else (t - 1) % RING
            _step(nc, pools, consts, t, prev, t % RING, sWD, sUD,
                  c_tile, first_step=(t == 0), skip_main=False,
                  mmdt=mmdt, hsplit=hsplit, csplit=csplit, parity=(t_enc + t) % 2)
            if t % WIN == WIN - 1:
                _ys_window(nc, pools, consts, t // WIN, dY)
            elif t == t_dec - 1:
                _ys_window(nc, pools, consts, t // WIN, dY, nsteps=(t % WIN) + 1)

    if not nc.is_finalized():
        nc.finalize()
    return nc


def prep_core_inputs(x_core, weights, mm_dtype="bf16", t_len=T):
    """Host-side layout prep for one core. x_core: [BL, t_len, 1] fp32."""
    npdt = ml_dtypes.bfloat16 if mm_dtype == "bf16" else np.float32
    out = {}
    xt = np.zeros((P, t_len * BL), dtype=np.float32)
    xt[0] = x_core[:, :, 0].T.reshape(-1)  # t-major: idx = t*BL + b
    xt[1] = 1.0
    out["XT"] = xt.astype(npdt)
    for tag, Wih, Whh, bih, bhh in (
        ("E", weights["enc_Wih"], weights["enc_Whh"], weights["enc_bih"], weights["enc_bhh"]),
        ("D", weights["dec_Wih"], weights["dec_Whh"], weights["dec_bih"], weights["dec_bhh"]),
    ):
        # W[k][kk, n] = Whh[n, 128k+kk]  (n = torch row order i,f,g,o).
        # g-gate rows are PRE-SCALED by 2: tanh(g) = 2*sigmoid(2g) - 1 lets
        # the g gate share the big sigmoid op with i and f.
        WT = np.ascontiguousarray(Whh.T).copy()  # [H, 4H]
        WT[:, 2 * H : 3 * H] *= 2.0
        out["W" + tag] = WT.reshape(KC, P, NT * P).astype(npdt)
        u = np.zeros((P, NT * P), dtype=np.float32)
        u[0] = Wih[:, 0]
        u[1] = bih + bhh
        u[:, 2 * H : 3 * H] *= 2.0
        out["U" + tag] = u.astype(npdt)
    out["LW"] = np.ascontiguousarray(weights["lin_W"][0].reshape(KC, P).T).astype(npdt)
    out["HF"] = np.full((P, P), 0.5, dtype=np.float32)
    return out


_CACHE = {}
_LAST_RESULTS = None


def kernel(**inputs) -> np.ndarray:
    global _LAST_RESULTS
    mm_dtype = "bf16"
    key = ("full", mm_dtype)
    if key not in _CACHE:
        _CACHE[key] = build_nc(T, T, mm_dtype)
    nc = _CACHE[key]

    x = np.asarray(inputs["x"], dtype=np.float32)
    in_maps = [
        prep_core_inputs(x[i * BL : (i + 1) * BL], inputs, mm_dtype)
        for i in range(N_CORES)
    ]

    res = run_bass_kernel_spmd(nc, in_maps, core_ids=list(range(N_CORES)))
    _LAST_RESULTS = res
    lin_b = float(np.asarray(inputs["lin_b"]).reshape(-1)[0])
    y = np.empty((B, T, 1), dtype=np.float32)
    for i in range(N_CORES):
        yi = np.asarray(res.results[i]["Y"], dtype=np.float32).reshape(T, BL)
        y[i * BL : (i + 1) * BL, :, 0] = yi.T + lin_b
    return y
